# revision 4
# baseline (speedup 1.0000x reference)
"""DangoCutouts Trainium2 kernel — two-pass PE-matmul resampler.

Computes reference:
    out[16, 3, 512, 512] =
      [full, gray(full), flip(full), gray(flip(full)), inner_0..11]
    where full = bilinear_resize(img, 4096 -> 512),
          inner_k = bilinear_resize(img[offy_k:+s_k, offx_k:+s_k] -> 512),
          inner_0 additionally grayscaled.

Strategy (8 NeuronCores, data-parallel over output rows):
  Core c computes output rows [64c, 64c+64) of all 16 outputs.
  13 distinct resamples (full + 12 inner). Per resample, per core:
    1. Row gather (dma_gather, SWDGE): T[128, 3, w_al] f32 where
       partition p = y-slot (p<64: y0-row of out row p; p>=64: y1-row),
       free q-slot = channel. One gather per resample (elem = w_al).
    2. Pass 1 (PE, fp32): per 128-wide x-chunk (127-stride so each
       output column's source pair stays within one chunk):
         out1[x, (c,i)] = T[:, c, chunk]^T @ Wy   (Wy = [y-slot, i]
       bilinear row weights, block-diagonal).  This fuses the row
       lerp and transposes x onto partitions in one step.
    3. out1 PSUM -> SBUF bf16 copy (ACT).
    4. Pass 2 (PE, bf16): final[(c,i), j] += out1_chunk^T @ Wx_chunk
       where Wx_chunk[x, j] holds (1-wx_j)/wx_j at the source pair
       rows.  Each j is written by exactly one chunk (no PSUM
       accumulation).  The overview resample additionally runs a
       reversed-Wx pass for the flip.
    5. Final PSUM -> SBUF copies, gray variants (DVE), DMA out.

  All gather/weight tables are host-built from the exact float32
  reference arithmetic; wx weights are bf16 (rel tolerance is 2e-2).
"""
import os
import numpy as np
import ml_dtypes

CUT = 512
H = W = 4096
GRAY_W = (0.2989, 0.587, 0.114)
N_INNER = 12
NSPEC = 13          # full + 12 inner
STRIP = 64          # output rows per core
NCORES = 8
SINGLE_PACKET = True
CHUNK_STRIDE = 127  # lhsT x-chunk stride (128-wide chunks, 1 overlap)

_CACHE = {}


# --------------------------------------------------------------------------
# host-side parameter math (replicates reference._crop_resize in float32)
# --------------------------------------------------------------------------

def _bilinear_params(offy, offx, size):
    s = np.float32(size)
    t = (np.arange(CUT, dtype=np.float32) + np.float32(0.5)) * s / np.float32(CUT) \
        - np.float32(0.5)
    y = np.clip(np.float32(offy) + t, np.float32(offy), np.float32(offy) + s - np.float32(1.0))
    x = np.clip(np.float32(offx) + t, np.float32(offx), np.float32(offx) + s - np.float32(1.0))
    y0 = np.floor(y).astype(np.int32)
    x0 = np.floor(x).astype(np.int32)
    y1 = np.minimum(y0 + 1, np.int32(offy) + np.int32(size) - 1)
    x1 = np.minimum(x0 + 1, np.int32(offx) + np.int32(size) - 1)
    wy = (y - y0.astype(np.float32)).astype(np.float32)
    wx = (x - x0.astype(np.float32)).astype(np.float32)
    # match XLA gather out-of-bounds clamp / negative wrap for degenerate inputs
    for a in (y0, y1):
        np.copyto(a, np.where(a < 0, a % H, np.minimum(a, H - 1)))
    for a in (x0, x1):
        np.copyto(a, np.where(a < 0, a % W, np.minimum(a, W - 1)))
    return y0, y1, wy, x0, x1, wx


def _col_window(x0, x1):
    cx0 = int(x0[0])
    w = int(x1[-1]) - cx0 + 1
    w_al = min(max((w + 63) // 64 * 64, 128), W)
    if cx0 + w_al > W:
        cx0 = W - w_al
    return cx0, w_al


def _wrap16(idx):
    """dma_gather idx-table layout: flat idx list wrapped 16-wide,
    replicated across the 8 Q7 cores."""
    idx = np.asarray(idx, np.int16)
    n = len(idx)
    assert n % 16 == 0
    cols = n // 16
    tile = np.zeros((128, cols), np.int16)
    blk = idx.reshape(cols, 16).T
    for g in range(8):
        tile[16 * g:16 * g + 16, :] = blk
    return tile


def _specs_from_inputs(sizes, offy, offx):
    specs = [(0, 0, min(H, W))]
    for k in range(N_INNER):
        specs.append((int(offy[k]), int(offx[k]), max(int(sizes[k]), 0)))
    return specs


def _params(specs):
    out = []
    for (oy, ox, s) in specs:
        y0, y1, wy, x0, x1, wx = _bilinear_params(oy, ox, max(s, 1) if s <= 0 else s)
        cx0, w_al = _col_window(x0, x1)
        out.append(dict(y0=y0, y1=y1, wy=wy, x0=x0, x1=x1, wx=wx, cx0=cx0, w_al=w_al))
    return out


# --------------------------------------------------------------------------
# column-pass chunk plan + weight tables
# --------------------------------------------------------------------------

def _chunk_plan(x0_rel, x1_rel, wx, w_al):
    """Group output columns j into 128-wide lhsT chunks (stride 127) such
    that both bilinear sources of each j live inside its chunk.  Returns
    (chunks, tables): chunks = [(x_k, j0, bw)], tables = [np[128, bw] f32]."""
    n = len(x0_rel)
    xk_of = np.minimum(CHUNK_STRIDE * (x0_rel // CHUNK_STRIDE), w_al - 128)
    xk_of = np.maximum(xk_of, 0)
    chunks, tables = [], []
    j = 0
    while j < n:
        xk = int(xk_of[j])
        j0 = j
        while j < n and int(xk_of[j]) == xk:
            j += 1
        bw = j - j0
        Wt = np.zeros((128, bw), np.float32)
        for t in range(j0, j):
            x0 = int(x0_rel[t]); x1 = int(x1_rel[t]); w = float(wx[t])
            lx0 = x0 - xk
            assert 0 <= lx0 < 128, (x0, xk)
            Wt[lx0, t - j0] += np.float32(1.0) - np.float32(w)
            if x1 == x0 + 1:
                assert x1 - xk < 128
                Wt[x1 - xk, t - j0] += np.float32(w)
            else:
                assert x1 == x0, (x0, x1)
                Wt[lx0, t - j0] += np.float32(w)
        chunks.append((xk, j0, bw))
        tables.append(Wt)
    return chunks, tables


def _wx_geom(params):
    """Build the (hashable) program geometry + the shared bf16 Wx table."""
    all_tables = []
    geom = []
    coff = 0
    for r, p in enumerate(params):
        w_al = p["w_al"]
        gx0 = (p["x0"] - p["cx0"]).astype(np.int64)
        gx1 = (p["x1"] - p["cx0"]).astype(np.int64)
        chunks, tables = _chunk_plan(gx0, gx1, p["wx"], w_al)
        entries = []
        for (xk, j0, bw), Wt in zip(chunks, tables):
            entries.append([xk, j0, bw, coff])
            all_tables.append(Wt)
            coff += bw
        if r == 0:
            # flip pass: same chunk set, reversed j mapping
            fchunks, ftables = _chunk_plan(gx0[::-1], gx1[::-1],
                                           p["wx"][::-1], w_al)
            fmap = {}
            for (xk, j0, bw), Wt in zip(fchunks, ftables):
                fmap[xk] = (j0, bw, coff)
                all_tables.append(Wt)
                coff += bw
            assert set(fmap) == {e[0] for e in entries}
            entries = [e + list(fmap[e[0]]) for e in entries]
        geom.append((p["cx0"], w_al, tuple(tuple(e) for e in entries)))
    wxt = np.concatenate(all_tables, axis=1).astype(ml_dtypes.bfloat16)
    return tuple(geom), wxt


def _core_tables(params, core):
    """Per-core row-gather idx table [128, 13*24] i16 and Wy [128, 13*64] f32."""
    r0 = core * STRIP
    ridx_cols, wy_cols = [], []
    ar = np.arange(STRIP)
    for p in params:
        y0s = p["y0"][r0:r0 + STRIP].astype(np.int64)
        y1s = p["y1"][r0:r0 + STRIP].astype(np.int64)
        idx = np.zeros(384, np.int64)
        for c in range(3):
            idx[c * 128:c * 128 + 64] = c * H + y0s
            idx[c * 128 + 64:c * 128 + 128] = c * H + y1s
        ridx_cols.append(_wrap16(idx))
        wys = p["wy"][r0:r0 + STRIP].astype(np.float32)
        Wy = np.zeros((128, STRIP), np.float32)
        Wy[ar, ar] = np.float32(1.0) - wys
        Wy[ar + STRIP, ar] = wys
        wy_cols.append(Wy)
    return (np.concatenate(ridx_cols, axis=1),
            np.concatenate(wy_cols, axis=1).astype(np.float32))


def _prepare(img, specs):
    params = _params(specs)
    geom, wxt = _wx_geom(params)
    in_maps = []
    for core in range(NCORES):
        ridx_all, wyt_all = _core_tables(params, core)
        in_maps.append({
            "img": img,
            "ridx": ridx_all,
            "wyt": wyt_all,
            "wxt": wxt,
        })
    return geom, in_maps


# --------------------------------------------------------------------------
# device program
# --------------------------------------------------------------------------

def _build_bass(geom, reps=1):
    import concourse.bacc as bacc
    import concourse.mybir as mybir
    from concourse.tile import TileContext

    f32 = mybir.dt.float32
    bf16 = mybir.dt.bfloat16
    i16 = mybir.dt.int16
    MUL = mybir.AluOpType.mult
    ADD = mybir.AluOpType.add

    wxcols = max(e[3] + e[2] for (_, _, entries) in geom for e in entries)
    wxcols = max(wxcols,
                 max((e[6] + e[5] for (_, _, es) in geom for e in es
                      if len(e) > 4), default=0))

    nc = bacc.Bacc("TRN2", target_bir_lowering=False)

    img = nc.dram_tensor("img", [3, H, W], f32, kind="ExternalInput")
    img_rows = img.rearrange("c h w -> (c h) w")
    ridx = nc.dram_tensor("ridx", [128, NSPEC * 24], i16, kind="ExternalInput")
    wyt = nc.dram_tensor("wyt", [128, NSPEC * 64], f32, kind="ExternalInput")
    wxt = nc.dram_tensor("wxt", [128, wxcols], bf16, kind="ExternalInput")

    out_d = nc.dram_tensor("out", [16, 3, STRIP, CUT], f32, kind="ExternalOutput")
    out_rows = out_d.rearrange("k c i j -> (k c i) j")

    def out_ap(k, c, nch=1):
        base = (k * 3 + c) * STRIP
        return out_rows[base:base + nch * STRIP, :]

    with TileContext(nc) as tc:
        with (
            tc.tile_pool(name="const", bufs=1) as cpool,
            tc.tile_pool(name="tdata", bufs=2) as tpool,
            tc.tile_pool(name="u", bufs=3) as upool,
            tc.tile_pool(name="otiles", bufs=2) as opool,
            tc.tile_pool(name="p1", bufs=2, space="PSUM") as p1pool,
            tc.tile_pool(name="fin", bufs=2, space="PSUM") as fpool,
            tc.tile_pool(name="flip", bufs=1, space="PSUM") as flpool,
        ):
            ridx_t = cpool.tile([128, NSPEC * 24], i16)
            nc.sync.dma_start(out=ridx_t[:], in_=ridx[:])
            wyt_t = cpool.tile([128, NSPEC * 64], f32)
            nc.sync.dma_start(out=wyt_t[:], in_=wyt[:])
            wxt_t = cpool.tile([128, wxcols], bf16)
            nc.sync.dma_start(out=wxt_t[:], in_=wxt[:])

            def writeout(O01, O2, kout, gray=False, gray_only=False):
                if not gray_only:
                    nc.sync.dma_start(out=out_ap(kout, 0, nch=2), in_=O01[:])
                    nc.sync.dma_start(out=out_ap(kout, 2), in_=O2[:])
                if gray or gray_only:
                    kg = kout + 1 if not gray_only else kout
                    ch1 = opool.tile([64, CUT], f32, tag="ch1")
                    nc.scalar.copy(out=ch1[:], in_=O01[64:128, :])
                    g = opool.tile([64, CUT], f32, tag="gray")
                    nc.scalar.mul(out=g[:], in_=O01[:64, :], mul=float(GRAY_W[0]))
                    nc.vector.scalar_tensor_tensor(out=g[:], in0=ch1[:],
                                                   scalar=float(GRAY_W[1]), in1=g[:],
                                                   op0=MUL, op1=ADD)
                    nc.vector.scalar_tensor_tensor(out=g[:], in0=O2[:],
                                                   scalar=float(GRAY_W[2]), in1=g[:],
                                                   op0=MUL, op1=ADD)
                    for c in range(3):
                        nc.sync.dma_start(out=out_ap(kg, c), in_=g[:])

            for _rep in range(reps):
                for r, (cx0, w_al, entries) in enumerate(geom):
                    T = tpool.tile([128, 3, w_al], f32, tag="T")
                    nc.gpsimd.dma_gather(
                        out_ap=T[:],
                        in_ap=img_rows[:, cx0:cx0 + w_al],
                        idxs_ap=ridx_t[:, r * 24:r * 24 + 24],
                        num_idxs=384,
                        num_idxs_reg=384,
                        elem_size=w_al,
                        elem_step=W,
                        single_packet=SINGLE_PACKET,
                    )
                    finalP = fpool.tile([128, CUT], f32, space="PSUM", tag="fP")
                    finalP2 = fpool.tile([64, CUT], f32, space="PSUM", tag="fP2")
                    is_ovw = len(entries[0]) > 4
                    if is_ovw:
                        flipP = flpool.tile([128, CUT], f32, space="PSUM", tag="xP")
                        flipP2 = flpool.tile([64, CUT], f32, space="PSUM", tag="xP2")
                    wy_sl = wyt_t[:, r * STRIP:(r + 1) * STRIP]

                    for ent in entries:
                        xk, j0, bw, coff = ent[:4]
                        P1 = p1pool.tile([128, 192], f32, space="PSUM", tag="P1")
                        for c in range(3):
                            nc.tensor.matmul(out=P1[:, 64 * c:64 * (c + 1)],
                                             lhsT=T[:, c, xk:xk + 128],
                                             rhs=wy_sl, start=True, stop=True)
                        U = upool.tile([128, 192], bf16, tag="U")
                        nc.scalar.copy(out=U[:], in_=P1[:])
                        nc.tensor.matmul(out=finalP[:, j0:j0 + bw],
                                         lhsT=U[:, 0:128],
                                         rhs=wxt_t[:, coff:coff + bw],
                                         start=True, stop=True)
                        nc.tensor.matmul(out=finalP2[:, j0:j0 + bw],
                                         lhsT=U[:, 128:192],
                                         rhs=wxt_t[:, coff:coff + bw],
                                         start=True, stop=True)
                        if is_ovw:
                            fj0, fbw, fcoff = ent[4:]
                            nc.tensor.matmul(out=flipP[:, fj0:fj0 + fbw],
                                             lhsT=U[:, 0:128],
                                             rhs=wxt_t[:, fcoff:fcoff + fbw],
                                             start=True, stop=True)
                            nc.tensor.matmul(out=flipP2[:, fj0:fj0 + fbw],
                                             lhsT=U[:, 128:192],
                                             rhs=wxt_t[:, fcoff:fcoff + fbw],
                                             start=True, stop=True)

                    O01 = opool.tile([128, CUT], f32, tag="O01")
                    O2 = opool.tile([64, CUT], f32, tag="O2")
                    nc.scalar.copy(out=O01[:], in_=finalP[:])
                    nc.scalar.copy(out=O2[:], in_=finalP2[:])
                    if r == 0:
                        writeout(O01, O2, 0, gray=True)
                        F01 = opool.tile([128, CUT], f32, tag="F01")
                        F2 = opool.tile([64, CUT], f32, tag="F2")
                        nc.scalar.copy(out=F01[:], in_=flipP[:])
                        nc.scalar.copy(out=F2[:], in_=flipP2[:])
                        writeout(F01, F2, 2, gray=True)
                    else:
                        writeout(O01, O2, 3 + r, gray_only=(r == 1))
    return nc


# --------------------------------------------------------------------------
# entry point
# --------------------------------------------------------------------------

def _run(img, specs, trace=False):
    from concourse.bass_utils import run_bass_kernel_spmd

    geom, in_maps = _prepare(img, specs)

    if geom in _CACHE:
        nc = _CACHE[geom]
    else:
        nc = _build_bass(geom)
        nc.compile()
        _CACHE[geom] = nc

    r = run_bass_kernel_spmd(nc, in_maps, core_ids=list(range(NCORES)),
                             trace=trace)
    strips = [r.results[c]["out"] for c in range(NCORES)]
    out = np.concatenate(strips, axis=2)
    return out, r


def kernel(**inputs):
    img = np.ascontiguousarray(np.asarray(inputs["input"], np.float32)[0])
    sizes = np.asarray(inputs["sizes"])
    offy = np.asarray(inputs["offy"])
    offx = np.asarray(inputs["offx"])
    specs = _specs_from_inputs(sizes, offy, offx)
    out, _ = _run(img, specs, trace=bool(int(os.environ.get("KERNEL_TRACE", "0"))))
    return out.astype(np.float32)


# revision 6
# speedup vs baseline: 3.3949x; 3.3949x over previous
"""DangoCutouts Trainium2 kernel — two-pass PE-matmul resampler.

Computes reference:
    out[16, 3, 512, 512] =
      [full, gray(full), flip(full), gray(flip(full)), inner_0..11]
    where full = bilinear_resize(img, 4096 -> 512),
          inner_k = bilinear_resize(img[offy_k:+s_k, offx_k:+s_k] -> 512),
          inner_0 additionally grayscaled.

Strategy (8 NeuronCores, data-parallel over output rows):
  Core c computes output rows [64c, 64c+64) of all 16 outputs.
  13 distinct resamples (full + 12 inner). Per resample, per core:
    1. Row gather (dma_gather, SWDGE): T[128, 3, w_al] f32 where
       partition p = y-slot (p<64: y0-row of out row p; p>=64: y1-row),
       free q-slot = channel. One gather per resample (elem = w_al).
    2. Pass 1 (PE, fp32): per 128-wide x-chunk (127-stride so each
       output column's source pair stays within one chunk):
         out1[x, (c,i)] = T[:, c, chunk]^T @ Wy   (Wy = [y-slot, i]
       bilinear row weights, block-diagonal).  This fuses the row
       lerp and transposes x onto partitions in one step.
    3. out1 PSUM -> SBUF bf16 copy (ACT).
    4. Pass 2 (PE, bf16): final[(c,i), j] += out1_chunk^T @ Wx_chunk
       where Wx_chunk[x, j] holds (1-wx_j)/wx_j at the source pair
       rows.  Each j is written by exactly one chunk (no PSUM
       accumulation).  The overview resample additionally runs a
       reversed-Wx pass for the flip.
    5. Final PSUM -> SBUF copies, gray variants (DVE), DMA out.

  All gather/weight tables are host-built from the exact float32
  reference arithmetic; wx weights are bf16 (rel tolerance is 2e-2).
"""
import os
import numpy as np
import ml_dtypes

CUT = 512
H = W = 4096
GRAY_W = (0.2989, 0.587, 0.114)
N_INNER = 12
NSPEC = 13          # full + 12 inner
STRIP = 64          # output rows per core
NCORES = 8
SINGLE_PACKET = True
CHUNK_STRIDE = 127  # lhsT x-chunk stride (128-wide chunks, 1 overlap)

_CACHE = {}


# --------------------------------------------------------------------------
# host-side parameter math (replicates reference._crop_resize in float32)
# --------------------------------------------------------------------------

def _bilinear_params(offy, offx, size):
    s = np.float32(size)
    t = (np.arange(CUT, dtype=np.float32) + np.float32(0.5)) * s / np.float32(CUT) \
        - np.float32(0.5)
    y = np.clip(np.float32(offy) + t, np.float32(offy), np.float32(offy) + s - np.float32(1.0))
    x = np.clip(np.float32(offx) + t, np.float32(offx), np.float32(offx) + s - np.float32(1.0))
    y0 = np.floor(y).astype(np.int32)
    x0 = np.floor(x).astype(np.int32)
    y1 = np.minimum(y0 + 1, np.int32(offy) + np.int32(size) - 1)
    x1 = np.minimum(x0 + 1, np.int32(offx) + np.int32(size) - 1)
    wy = (y - y0.astype(np.float32)).astype(np.float32)
    wx = (x - x0.astype(np.float32)).astype(np.float32)
    # match XLA gather out-of-bounds clamp / negative wrap for degenerate inputs
    for a in (y0, y1):
        np.copyto(a, np.where(a < 0, a % H, np.minimum(a, H - 1)))
    for a in (x0, x1):
        np.copyto(a, np.where(a < 0, a % W, np.minimum(a, W - 1)))
    return y0, y1, wy, x0, x1, wx


def _col_window(x0, x1):
    cx0 = int(x0[0])
    w = int(x1[-1]) - cx0 + 1
    w_al = min(max((w + 63) // 64 * 64, 128), W)
    if cx0 + w_al > W:
        cx0 = W - w_al
    return cx0, w_al


def _wrap16(idx):
    """dma_gather idx-table layout: flat idx list wrapped 16-wide,
    replicated across the 8 Q7 cores."""
    idx = np.asarray(idx, np.int16)
    n = len(idx)
    assert n % 16 == 0
    cols = n // 16
    tile = np.zeros((128, cols), np.int16)
    blk = idx.reshape(cols, 16).T
    for g in range(8):
        tile[16 * g:16 * g + 16, :] = blk
    return tile


def _specs_from_inputs(sizes, offy, offx):
    specs = [(0, 0, min(H, W))]
    for k in range(N_INNER):
        specs.append((int(offy[k]), int(offx[k]), max(int(sizes[k]), 0)))
    return specs


def _params(specs):
    out = []
    for (oy, ox, s) in specs:
        y0, y1, wy, x0, x1, wx = _bilinear_params(oy, ox, max(s, 1) if s <= 0 else s)
        cx0, w_al = _col_window(x0, x1)
        out.append(dict(y0=y0, y1=y1, wy=wy, x0=x0, x1=x1, wx=wx, cx0=cx0, w_al=w_al))
    return out


# --------------------------------------------------------------------------
# column-pass chunk plan + weight tables
# --------------------------------------------------------------------------

def _chunk_plan(x0_rel, x1_rel, wx, w_al):
    """Group output columns j into 128-wide lhsT chunks (stride 127) such
    that both bilinear sources of each j live inside its chunk.  Returns
    (chunks, tables): chunks = [(x_k, j0, bw)], tables = [np[128, bw] f32]."""
    n = len(x0_rel)
    xk_of = np.minimum(CHUNK_STRIDE * (x0_rel // CHUNK_STRIDE), w_al - 128)
    xk_of = np.maximum(xk_of, 0)
    chunks, tables = [], []
    j = 0
    while j < n:
        xk = int(xk_of[j])
        j0 = j
        while j < n and int(xk_of[j]) == xk:
            j += 1
        bw = j - j0
        Wt = np.zeros((128, bw), np.float32)
        for t in range(j0, j):
            x0 = int(x0_rel[t]); x1 = int(x1_rel[t]); w = float(wx[t])
            lx0 = x0 - xk
            assert 0 <= lx0 < 128, (x0, xk)
            Wt[lx0, t - j0] += np.float32(1.0) - np.float32(w)
            if x1 == x0 + 1:
                assert x1 - xk < 128
                Wt[x1 - xk, t - j0] += np.float32(w)
            else:
                assert x1 == x0, (x0, x1)
                Wt[lx0, t - j0] += np.float32(w)
        chunks.append((xk, j0, bw))
        tables.append(Wt)
    return chunks, tables


def _wx_geom(params):
    """Build the (hashable) program geometry + the shared bf16 Wx table."""
    all_tables = []
    geom = []
    coff = 0
    for r, p in enumerate(params):
        w_al = p["w_al"]
        gx0 = (p["x0"] - p["cx0"]).astype(np.int64)
        gx1 = (p["x1"] - p["cx0"]).astype(np.int64)
        chunks, tables = _chunk_plan(gx0, gx1, p["wx"], w_al)
        entries = []
        for (xk, j0, bw), Wt in zip(chunks, tables):
            entries.append([xk, j0, bw, coff])
            all_tables.append(Wt)
            coff += bw
        if r == 0:
            # flip pass: same chunk set, reversed j mapping
            fchunks, ftables = _chunk_plan(gx0[::-1], gx1[::-1],
                                           p["wx"][::-1], w_al)
            fmap = {}
            for (xk, j0, bw), Wt in zip(fchunks, ftables):
                fmap[xk] = (j0, bw, coff)
                all_tables.append(Wt)
                coff += bw
            assert set(fmap) == {e[0] for e in entries}
            entries = [e + list(fmap[e[0]]) for e in entries]
        geom.append((p["cx0"], w_al, tuple(tuple(e) for e in entries)))
    wxt = np.concatenate(all_tables, axis=1).astype(ml_dtypes.bfloat16)
    return tuple(geom), wxt


def _core_tables(params, core):
    """Per-core row-gather idx table [128, 13*24] i16 and Wy [128, 13*64] f32."""
    r0 = core * STRIP
    ridx_cols, wy_cols = [], []
    ar = np.arange(STRIP)
    for p in params:
        y0s = p["y0"][r0:r0 + STRIP].astype(np.int64)
        y1s = p["y1"][r0:r0 + STRIP].astype(np.int64)
        idx = np.zeros(384, np.int64)
        for c in range(3):
            idx[c * 128:c * 128 + 64] = c * H + y0s
            idx[c * 128 + 64:c * 128 + 128] = c * H + y1s
        ridx_cols.append(_wrap16(idx))
        wys = p["wy"][r0:r0 + STRIP].astype(np.float32)
        Wy = np.zeros((128, STRIP), np.float32)
        Wy[ar, ar] = np.float32(1.0) - wys
        Wy[ar + STRIP, ar] = wys
        wy_cols.append(Wy)
    return (np.concatenate(ridx_cols, axis=1),
            np.concatenate(wy_cols, axis=1).astype(np.float32))


def _prepare(img, specs):
    params = _params(specs)
    geom, wxt = _wx_geom(params)
    in_maps = []
    for core in range(NCORES):
        ridx_all, wyt_all = _core_tables(params, core)
        in_maps.append({
            "img": img,
            "ridx": ridx_all,
            "wyt": wyt_all,
            "wxt": wxt,
        })
    return geom, in_maps


# --------------------------------------------------------------------------
# device program
# --------------------------------------------------------------------------

def _build_bass(geom, reps=1, ablate=None):
    import concourse.bacc as bacc
    import concourse.mybir as mybir
    from concourse.tile import TileContext

    f32 = mybir.dt.float32
    bf16 = mybir.dt.bfloat16
    i16 = mybir.dt.int16
    MUL = mybir.AluOpType.mult
    ADD = mybir.AluOpType.add

    wxcols = max(e[3] + e[2] for (_, _, entries) in geom for e in entries)
    wxcols = max(wxcols,
                 max((e[6] + e[5] for (_, _, es) in geom for e in es
                      if len(e) > 4), default=0))

    nc = bacc.Bacc("TRN2", target_bir_lowering=False)

    img = nc.dram_tensor("img", [3, H, W], f32, kind="ExternalInput")
    img_rows = img.rearrange("c h w -> (c h) w")
    ridx = nc.dram_tensor("ridx", [128, NSPEC * 24], i16, kind="ExternalInput")
    wyt = nc.dram_tensor("wyt", [128, NSPEC * 64], f32, kind="ExternalInput")
    wxt = nc.dram_tensor("wxt", [128, wxcols], bf16, kind="ExternalInput")

    out_d = nc.dram_tensor("out", [16, 3, STRIP, CUT], f32, kind="ExternalOutput")
    out_rows = out_d.rearrange("k c i j -> (k c i) j")

    def out_ap(k, c, nch=1):
        base = (k * 3 + c) * STRIP
        return out_rows[base:base + nch * STRIP, :]

    with TileContext(nc) as tc:
        with (
            tc.tile_pool(name="const", bufs=1) as cpool,
            tc.tile_pool(name="tdata", bufs=2) as tpool,
            tc.tile_pool(name="u", bufs=3) as upool,
            tc.tile_pool(name="otiles", bufs=2) as opool,
            tc.tile_pool(name="p1", bufs=2, space="PSUM") as p1pool,
            tc.tile_pool(name="fin", bufs=2, space="PSUM") as fpool,
            tc.tile_pool(name="flip", bufs=1, space="PSUM") as flpool,
        ):
            ridx_t = cpool.tile([128, NSPEC * 24], i16)
            nc.sync.dma_start(out=ridx_t[:], in_=ridx[:])
            wyt_t = cpool.tile([128, NSPEC * 64], f32)
            nc.sync.dma_start(out=wyt_t[:], in_=wyt[:])
            wxt_t = cpool.tile([128, wxcols], bf16)
            nc.sync.dma_start(out=wxt_t[:], in_=wxt[:])

            def writeout(O01, O2, kout, gray=False, gray_only=False):
                if not gray_only:
                    nc.sync.dma_start(out=out_ap(kout, 0, nch=2), in_=O01[:])
                    nc.sync.dma_start(out=out_ap(kout, 2), in_=O2[:])
                if gray or gray_only:
                    kg = kout + 1 if not gray_only else kout
                    ch1 = opool.tile([64, CUT], f32, tag="ch1")
                    nc.scalar.copy(out=ch1[:], in_=O01[64:128, :])
                    g = opool.tile([64, CUT], f32, tag="gray")
                    nc.scalar.mul(out=g[:], in_=O01[:64, :], mul=float(GRAY_W[0]))
                    nc.vector.scalar_tensor_tensor(out=g[:], in0=ch1[:],
                                                   scalar=float(GRAY_W[1]), in1=g[:],
                                                   op0=MUL, op1=ADD)
                    nc.vector.scalar_tensor_tensor(out=g[:], in0=O2[:],
                                                   scalar=float(GRAY_W[2]), in1=g[:],
                                                   op0=MUL, op1=ADD)
                    for c in range(3):
                        nc.sync.dma_start(out=out_ap(kg, c), in_=g[:])

            PIPE = 2   # chunks of pass1 issued ahead of each pass2

            for _rep in range(reps):
                for r, (cx0, w_al, entries) in enumerate(geom):
                    T = tpool.tile([128, 3, w_al], f32, tag="T")
                    if ablate == "hwdma":
                        img_hcw = img.rearrange("c h w -> h c w")
                        nc.sync.dma_start(
                            out=T[:], in_=img_hcw[0:128, :, cx0:cx0 + w_al])
                    else:
                        nc.gpsimd.dma_gather(
                            out_ap=T[:],
                            in_ap=img_rows[:, cx0:cx0 + w_al],
                            idxs_ap=ridx_t[:, r * 24:r * 24 + 24],
                            num_idxs=384,
                            num_idxs_reg=384,
                            elem_size=w_al,
                            elem_step=W,
                            single_packet=SINGLE_PACKET,
                        )
                    if ablate == "nocompute" and _rep > 0:
                        continue
                    finalP = fpool.tile([128, CUT], f32, space="PSUM", tag="fP")
                    finalP2 = fpool.tile([64, CUT], f32, space="PSUM", tag="fP2")
                    is_ovw = len(entries[0]) > 4
                    if is_ovw:
                        flipP = flpool.tile([128, CUT], f32, space="PSUM", tag="xP")
                        flipP2 = flpool.tile([64, CUT], f32, space="PSUM", tag="xP2")
                    wy_sl = wyt_t[:, r * STRIP:(r + 1) * STRIP]

                    def pass2(ent, U):
                        xk, j0, bw, coff = ent[:4]
                        nc.tensor.matmul(out=finalP[:, j0:j0 + bw],
                                         lhsT=U[:, 0:128],
                                         rhs=wxt_t[:, coff:coff + bw],
                                         start=True, stop=True)
                        nc.tensor.matmul(out=finalP2[:, j0:j0 + bw],
                                         lhsT=U[:, 128:192],
                                         rhs=wxt_t[:, coff:coff + bw],
                                         start=True, stop=True)
                        if is_ovw:
                            fj0, fbw, fcoff = ent[4:]
                            nc.tensor.matmul(out=flipP[:, fj0:fj0 + fbw],
                                             lhsT=U[:, 0:128],
                                             rhs=wxt_t[:, fcoff:fcoff + fbw],
                                             start=True, stop=True)
                            nc.tensor.matmul(out=flipP2[:, fj0:fj0 + fbw],
                                             lhsT=U[:, 128:192],
                                             rhs=wxt_t[:, fcoff:fcoff + fbw],
                                             start=True, stop=True)

                    staged = []
                    for ci, ent in enumerate(entries):
                        xk = ent[0]
                        P1 = p1pool.tile([128, 192], f32, space="PSUM", tag="P1")
                        for c in range(3):
                            nc.tensor.matmul(out=P1[:, 64 * c:64 * (c + 1)],
                                             lhsT=T[:, c, xk:xk + 128],
                                             rhs=wy_sl, start=True, stop=True)
                        U = upool.tile([128, 192], bf16, tag="U")
                        if ci % 2 == 0:
                            nc.scalar.copy(out=U[:], in_=P1[:])
                        else:
                            nc.vector.tensor_copy(U[:], P1[:])
                        staged.append((ent, U))
                        if len(staged) > PIPE:
                            pass2(*staged.pop(0))
                    for s in staged:
                        pass2(*s)

                    O01 = opool.tile([128, CUT], f32, tag="O01")
                    O2 = opool.tile([64, CUT], f32, tag="O2")
                    nc.scalar.copy(out=O01[:], in_=finalP[:])
                    nc.scalar.copy(out=O2[:], in_=finalP2[:])
                    if r == 0:
                        writeout(O01, O2, 0, gray=True)
                        F01 = opool.tile([128, CUT], f32, tag="F01")
                        F2 = opool.tile([64, CUT], f32, tag="F2")
                        nc.scalar.copy(out=F01[:], in_=flipP[:])
                        nc.scalar.copy(out=F2[:], in_=flipP2[:])
                        writeout(F01, F2, 2, gray=True)
                    else:
                        writeout(O01, O2, 3 + r, gray_only=(r == 1))
    return nc


# --------------------------------------------------------------------------
# entry point
# --------------------------------------------------------------------------

def _run(img, specs, trace=False):
    from concourse.bass_utils import run_bass_kernel_spmd

    geom, in_maps = _prepare(img, specs)

    if geom in _CACHE:
        nc = _CACHE[geom]
    else:
        nc = _build_bass(geom)
        nc.compile()
        _CACHE[geom] = nc

    r = run_bass_kernel_spmd(nc, in_maps, core_ids=list(range(NCORES)),
                             trace=trace)
    strips = [r.results[c]["out"] for c in range(NCORES)]
    out = np.concatenate(strips, axis=2)
    return out, r


def kernel(**inputs):
    img = np.ascontiguousarray(np.asarray(inputs["input"], np.float32)[0])
    sizes = np.asarray(inputs["sizes"])
    offy = np.asarray(inputs["offy"])
    offx = np.asarray(inputs["offx"])
    specs = _specs_from_inputs(sizes, offy, offx)
    out, _ = _run(img, specs, trace=bool(int(os.environ.get("KERNEL_TRACE", "0"))))
    return out.astype(np.float32)


# revision 15
# speedup vs baseline: 9.9336x; 2.9261x over previous
"""DangoCutouts Trainium2 kernel — two-pass PE-matmul resampler.

Computes reference:
    out[16, 3, 512, 512] =
      [full, gray(full), flip(full), gray(flip(full)), inner_0..11]
    where full = bilinear_resize(img, 4096 -> 512),
          inner_k = bilinear_resize(img[offy_k:+s_k, offx_k:+s_k] -> 512),
          inner_0 additionally grayscaled.

Strategy (8 NeuronCores, data-parallel over output rows):
  Core c computes output rows [64c, 64c+64) of all 16 outputs.
  13 distinct resamples (full + 12 inner). Per resample, per core:
    1. Row gather (dma_gather, SWDGE): T[128, 3, w_al] f32 where
       partition p = y-slot (p<64: y0-row of out row p; p>=64: y1-row),
       free q-slot = channel. One gather per resample (elem = w_al).
    2. Pass 1 (PE, fp32): per 128-wide x-chunk (127-stride so each
       output column's source pair stays within one chunk):
         out1[x, (c,i)] = T[:, c, chunk]^T @ Wy   (Wy = [y-slot, i]
       bilinear row weights, block-diagonal).  This fuses the row
       lerp and transposes x onto partitions in one step.
    3. out1 PSUM -> SBUF bf16 copy (ACT).
    4. Pass 2 (PE, bf16): final[(c,i), j] += out1_chunk^T @ Wx_chunk
       where Wx_chunk[x, j] holds (1-wx_j)/wx_j at the source pair
       rows.  Each j is written by exactly one chunk (no PSUM
       accumulation).  The overview resample additionally runs a
       reversed-Wx pass for the flip.
    5. Final PSUM -> SBUF copies, gray variants (DVE), DMA out.

  All gather/weight tables are host-built from the exact float32
  reference arithmetic; wx weights are bf16 (rel tolerance is 2e-2).
"""
import os
import numpy as np
import ml_dtypes

CUT = 512
H = W = 4096
GRAY_W = (0.2989, 0.587, 0.114)
N_INNER = 12
NSPEC = 13          # full + 12 inner
STRIP = 64          # output rows per core
NCORES = 8
SINGLE_PACKET = True
CHUNK_STRIDE = 127  # lhsT x-chunk stride (128-wide chunks, 1 overlap)

_CACHE = {}


# --------------------------------------------------------------------------
# host-side parameter math (replicates reference._crop_resize in float32)
# --------------------------------------------------------------------------

def _bilinear_params(offy, offx, size):
    s = np.float32(size)
    t = (np.arange(CUT, dtype=np.float32) + np.float32(0.5)) * s / np.float32(CUT) \
        - np.float32(0.5)
    y = np.clip(np.float32(offy) + t, np.float32(offy), np.float32(offy) + s - np.float32(1.0))
    x = np.clip(np.float32(offx) + t, np.float32(offx), np.float32(offx) + s - np.float32(1.0))
    y0 = np.floor(y).astype(np.int32)
    x0 = np.floor(x).astype(np.int32)
    y1 = np.minimum(y0 + 1, np.int32(offy) + np.int32(size) - 1)
    x1 = np.minimum(x0 + 1, np.int32(offx) + np.int32(size) - 1)
    wy = (y - y0.astype(np.float32)).astype(np.float32)
    wx = (x - x0.astype(np.float32)).astype(np.float32)
    # match XLA gather out-of-bounds clamp / negative wrap for degenerate inputs
    for a in (y0, y1):
        np.copyto(a, np.where(a < 0, a % H, np.minimum(a, H - 1)))
    for a in (x0, x1):
        np.copyto(a, np.where(a < 0, a % W, np.minimum(a, W - 1)))
    return y0, y1, wy, x0, x1, wx


def _col_window(x0, x1):
    cx0 = int(x0[0])
    w = int(x1[-1]) - cx0 + 1
    w_al = min(max((w + 63) // 64 * 64, 128), W)
    if cx0 + w_al > W:
        cx0 = W - w_al
    return cx0, w_al


def _wrap16(idx):
    """dma_gather idx-table layout: flat idx list wrapped 16-wide,
    replicated across the 8 Q7 cores."""
    idx = np.asarray(idx, np.int16)
    n = len(idx)
    assert n % 16 == 0
    cols = n // 16
    tile = np.zeros((128, cols), np.int16)
    blk = idx.reshape(cols, 16).T
    for g in range(8):
        tile[16 * g:16 * g + 16, :] = blk
    return tile


def _specs_from_inputs(sizes, offy, offx):
    specs = [(0, 0, min(H, W))]
    for k in range(N_INNER):
        specs.append((int(offy[k]), int(offx[k]), max(int(sizes[k]), 0)))
    return specs


def _params(specs):
    out = []
    for (oy, ox, s) in specs:
        y0, y1, wy, x0, x1, wx = _bilinear_params(oy, ox, max(s, 1) if s <= 0 else s)
        cx0, w_al = _col_window(x0, x1)
        out.append(dict(y0=y0, y1=y1, wy=wy, x0=x0, x1=x1, wx=wx, cx0=cx0, w_al=w_al))
    return out


# --------------------------------------------------------------------------
# column-pass chunk plan + weight tables
# --------------------------------------------------------------------------

def _chunk_plan(x0_rel, x1_rel, wx, w_al):
    """Group output columns j into 128-wide lhsT chunks (stride 127) such
    that both bilinear sources of each j live inside its chunk.  Returns
    (chunks, tables): chunks = [(x_k, j0, bw)], tables = [np[128, bw] f32]."""
    n = len(x0_rel)
    xk_of = np.minimum(CHUNK_STRIDE * (x0_rel // CHUNK_STRIDE), w_al - 128)
    xk_of = np.maximum(xk_of, 0)
    chunks, tables = [], []
    j = 0
    while j < n:
        xk = int(xk_of[j])
        j0 = j
        while j < n and int(xk_of[j]) == xk:
            j += 1
        bw = j - j0
        Wt = np.zeros((128, bw), np.float32)
        for t in range(j0, j):
            x0 = int(x0_rel[t]); x1 = int(x1_rel[t]); w = float(wx[t])
            lx0 = x0 - xk
            assert 0 <= lx0 < 128, (x0, xk)
            Wt[lx0, t - j0] += np.float32(1.0) - np.float32(w)
            if x1 == x0 + 1:
                assert x1 - xk < 128
                Wt[x1 - xk, t - j0] += np.float32(w)
            else:
                assert x1 == x0, (x0, x1)
                Wt[lx0, t - j0] += np.float32(w)
        chunks.append((xk, j0, bw))
        tables.append(Wt)
    return chunks, tables


def _wx_geom(params):
    """Build the (hashable) program geometry + the shared bf16 Wx table."""
    all_tables = []
    geom = []
    coff = 0
    for r, p in enumerate(params):
        w_al = p["w_al"]
        gx0 = (p["x0"] - p["cx0"]).astype(np.int64)
        gx1 = (p["x1"] - p["cx0"]).astype(np.int64)
        chunks, tables = _chunk_plan(gx0, gx1, p["wx"], w_al)
        entries = []
        for (xk, j0, bw), Wt in zip(chunks, tables):
            entries.append([xk, j0, bw, coff])
            all_tables.append(Wt)
            coff += bw
        if r == 0:
            # flip pass: same chunk set, reversed j mapping
            fchunks, ftables = _chunk_plan(gx0[::-1], gx1[::-1],
                                           p["wx"][::-1], w_al)
            fmap = {}
            for (xk, j0, bw), Wt in zip(fchunks, ftables):
                fmap[xk] = (j0, bw, coff)
                all_tables.append(Wt)
                coff += bw
            assert set(fmap) == {e[0] for e in entries}
            entries = [e + list(fmap[e[0]]) for e in entries]
        geom.append((p["cx0"], w_al, tuple(tuple(e) for e in entries)))
    wxt = np.concatenate(all_tables, axis=1).astype(ml_dtypes.bfloat16)
    return tuple(geom), wxt


def _core_tables(params, core):
    """Per-core row-gather idx table [128, 13*24] i16 and Wy [128, 13*64] f32."""
    r0 = core * STRIP
    ridx_cols, wy_cols = [], []
    ar = np.arange(STRIP)
    for p in params:
        y0s = p["y0"][r0:r0 + STRIP].astype(np.int64)
        y1s = p["y1"][r0:r0 + STRIP].astype(np.int64)
        idx = np.zeros(384, np.int64)
        for c in range(3):
            idx[c * 128:c * 128 + 64] = c * H + y0s
            idx[c * 128 + 64:c * 128 + 128] = c * H + y1s
        ridx_cols.append(_wrap16(idx))
        wys = p["wy"][r0:r0 + STRIP].astype(np.float32)
        Wy = np.zeros((128, STRIP), np.float32)
        Wy[ar, ar] = np.float32(1.0) - wys
        Wy[ar + STRIP, ar] = wys
        wy_cols.append(Wy)
    return (np.concatenate(ridx_cols, axis=1),
            np.concatenate(wy_cols, axis=1).astype(np.float32))


def _prepare(img, specs):
    params = _params(specs)
    geom, wxt = _wx_geom(params)
    in_maps = []
    for core in range(NCORES):
        ridx_all, wyt_all = _core_tables(params, core)
        in_maps.append({
            "img": img,
            "ridx": ridx_all,
            "wyt": wyt_all,
            "wxt": wxt,
        })
    return geom, in_maps


# --------------------------------------------------------------------------
# device program
# --------------------------------------------------------------------------

def _build_bass(geom, reps=1, ablate=None, pass1_bf16=False, pipe=2):
    import concourse.bacc as bacc
    import concourse.mybir as mybir
    from concourse.tile import TileContext

    f32 = mybir.dt.float32
    bf16 = mybir.dt.bfloat16
    i16 = mybir.dt.int16
    MUL = mybir.AluOpType.mult
    ADD = mybir.AluOpType.add

    wxcols = max(e[3] + e[2] for (_, _, entries) in geom for e in entries)
    wxcols = max(wxcols,
                 max((e[6] + e[5] for (_, _, es) in geom for e in es
                      if len(e) > 4), default=0))

    nc = bacc.Bacc("TRN2", target_bir_lowering=False)

    img = nc.dram_tensor("img", [3, H, W], f32, kind="ExternalInput")
    img_rows = img.rearrange("c h w -> (c h) w")
    ridx = nc.dram_tensor("ridx", [128, NSPEC * 24], i16, kind="ExternalInput")
    wyt = nc.dram_tensor("wyt", [128, NSPEC * 64], f32, kind="ExternalInput")
    wxt = nc.dram_tensor("wxt", [128, wxcols], bf16, kind="ExternalInput")

    out_d = nc.dram_tensor("out", [16, 3, STRIP, CUT], f32, kind="ExternalOutput")
    out_rows = out_d.rearrange("k c i j -> (k c i) j")

    def out_ap(k, c, nch=1):
        base = (k * 3 + c) * STRIP
        return out_rows[base:base + nch * STRIP, :]

    with TileContext(nc) as tc:
        with (
            tc.tile_pool(name="const", bufs=1) as cpool,
            tc.tile_pool(name="tdata", bufs=2) as tpool,
            tc.tile_pool(name="u", bufs=pipe + 2) as upool,
            tc.tile_pool(name="otiles", bufs=2) as opool,
            tc.tile_pool(name="p1", bufs=max(2, pipe), space="PSUM") as p1pool,
            tc.tile_pool(name="fin", bufs=(1 if pipe > 2 else 2), space="PSUM") as fpool,
            tc.tile_pool(name="flip", bufs=1, space="PSUM") as flpool,
        ):
            ridx_t = cpool.tile([128, NSPEC * 24], i16)
            nc.sync.dma_start(out=ridx_t[:], in_=ridx[:])
            wyt_t = cpool.tile([128, NSPEC * 64], f32)
            nc.sync.dma_start(out=wyt_t[:], in_=wyt[:])
            wxt_t = cpool.tile([128, wxcols], bf16)
            nc.sync.dma_start(out=wxt_t[:], in_=wxt[:])
            if pass1_bf16:
                wyt_b = cpool.tile([128, NSPEC * 64], bf16)
                nc.vector.tensor_copy(wyt_b[:], wyt_t[:])

            def writeout(O01, O2, kout, gray=False, gray_only=False):
                if not gray_only:
                    nc.sync.dma_start(out=out_ap(kout, 0, nch=2), in_=O01[:])
                    nc.sync.dma_start(out=out_ap(kout, 2), in_=O2[:])
                if gray or gray_only:
                    kg = kout + 1 if not gray_only else kout
                    ch1 = opool.tile([64, CUT], f32, tag="ch1")
                    nc.scalar.copy(out=ch1[:], in_=O01[64:128, :])
                    g = opool.tile([64, CUT], f32, tag="gray")
                    nc.scalar.mul(out=g[:], in_=O01[:64, :], mul=float(GRAY_W[0]))
                    nc.vector.scalar_tensor_tensor(out=g[:], in0=ch1[:],
                                                   scalar=float(GRAY_W[1]), in1=g[:],
                                                   op0=MUL, op1=ADD)
                    nc.vector.scalar_tensor_tensor(out=g[:], in0=O2[:],
                                                   scalar=float(GRAY_W[2]), in1=g[:],
                                                   op0=MUL, op1=ADD)
                    for c in range(3):
                        nc.sync.dma_start(out=out_ap(kg, c), in_=g[:])

            PIPE = pipe   # chunks of pass1 issued ahead of each pass2

            for _rep in range(reps):
                for r, (cx0, w_al, entries) in enumerate(geom):
                    T = tpool.tile([128, 3, w_al], f32, tag="T")
                    if ablate == "hwdma":
                        img_hcw = img.rearrange("c h w -> h c w")
                        nc.sync.dma_start(
                            out=T[:], in_=img_hcw[0:128, :, cx0:cx0 + w_al])
                    else:
                        nc.gpsimd.dma_gather(
                            out_ap=T[:],
                            in_ap=img_rows[:, cx0:cx0 + w_al],
                            idxs_ap=ridx_t[:, r * 24:r * 24 + 24],
                            num_idxs=384,
                            num_idxs_reg=384,
                            elem_size=w_al,
                            elem_step=W,
                            single_packet=SINGLE_PACKET,
                        )
                    if ablate == "nocompute" and _rep > 0:
                        continue
                    if pass1_bf16:
                        Tb = tpool.tile([128, 3, w_al], bf16, tag="Tb")
                        # split the f32->bf16 cast across DVE and ACT
                        nc.vector.tensor_copy(Tb[:, 0:2, :], T[:, 0:2, :])
                        nc.scalar.copy(out=Tb[:, 2, :], in_=T[:, 2, :])
                        Tmm = Tb
                    else:
                        Tmm = T
                    finalP = fpool.tile([128, CUT], f32, space="PSUM", tag="fP")
                    finalP2 = fpool.tile([64, CUT], f32, space="PSUM", tag="fP2")
                    is_ovw = len(entries[0]) > 4
                    if is_ovw:
                        flipP = flpool.tile([128, CUT], f32, space="PSUM", tag="xP")
                        flipP2 = flpool.tile([64, CUT], f32, space="PSUM", tag="xP2")
                    wy_sl = (wyt_b if pass1_bf16 else wyt_t)[:, r * STRIP:(r + 1) * STRIP]

                    def pass2(ent, U):
                        xk, j0, bw, coff = ent[:4]
                        nc.tensor.matmul(out=finalP[:, j0:j0 + bw],
                                         lhsT=U[:, 0:128],
                                         rhs=wxt_t[:, coff:coff + bw],
                                         start=True, stop=True)
                        nc.tensor.matmul(out=finalP2[:, j0:j0 + bw],
                                         lhsT=U[:, 128:192],
                                         rhs=wxt_t[:, coff:coff + bw],
                                         start=True, stop=True)
                        if is_ovw:
                            fj0, fbw, fcoff = ent[4:]
                            nc.tensor.matmul(out=flipP[:, fj0:fj0 + fbw],
                                             lhsT=U[:, 0:128],
                                             rhs=wxt_t[:, fcoff:fcoff + fbw],
                                             start=True, stop=True)
                            nc.tensor.matmul(out=flipP2[:, fj0:fj0 + fbw],
                                             lhsT=U[:, 128:192],
                                             rhs=wxt_t[:, fcoff:fcoff + fbw],
                                             start=True, stop=True)

                    staged = []
                    for ci, ent in enumerate(entries):
                        xk = ent[0]
                        P1 = p1pool.tile([128, 192], f32, space="PSUM", tag="P1")
                        for c in range(3):
                            nc.tensor.matmul(out=P1[:, 64 * c:64 * (c + 1)],
                                             lhsT=Tmm[:, c, xk:xk + 128],
                                             rhs=wy_sl, start=True, stop=True)
                        U = upool.tile([128, 192], bf16, tag="U")
                        if ci % 2 == 0:
                            nc.scalar.copy(out=U[:], in_=P1[:])
                        else:
                            nc.vector.tensor_copy(U[:], P1[:])
                        staged.append((ent, U))
                        if len(staged) > PIPE:
                            pass2(*staged.pop(0))
                    for s in staged:
                        pass2(*s)

                    O01 = opool.tile([128, CUT], f32, tag="O01")
                    O2 = opool.tile([64, CUT], f32, tag="O2")
                    nc.scalar.copy(out=O01[:], in_=finalP[:])
                    nc.scalar.copy(out=O2[:], in_=finalP2[:])
                    if r == 0:
                        writeout(O01, O2, 0, gray=True)
                        F01 = opool.tile([128, CUT], f32, tag="F01")
                        F2 = opool.tile([64, CUT], f32, tag="F2")
                        nc.scalar.copy(out=F01[:], in_=flipP[:])
                        nc.scalar.copy(out=F2[:], in_=flipP2[:])
                        writeout(F01, F2, 2, gray=True)
                    else:
                        writeout(O01, O2, 3 + r, gray_only=(r == 1))
    return nc


# --------------------------------------------------------------------------
# entry point
# --------------------------------------------------------------------------

def _run(img, specs, trace=False):
    from concourse.bass_utils import run_bass_kernel_spmd

    geom, in_maps = _prepare(img, specs)

    if geom in _CACHE:
        nc = _CACHE[geom]
    else:
        nc = _build_bass(geom)
        nc.compile()
        _CACHE[geom] = nc

    r = run_bass_kernel_spmd(nc, in_maps, core_ids=list(range(NCORES)),
                             trace=trace)
    strips = [r.results[c]["out"] for c in range(NCORES)]
    out = np.concatenate(strips, axis=2)
    return out, r


def kernel(**inputs):
    img = np.ascontiguousarray(np.asarray(inputs["input"], np.float32)[0])
    sizes = np.asarray(inputs["sizes"])
    offy = np.asarray(inputs["offy"])
    offx = np.asarray(inputs["offx"])
    specs = _specs_from_inputs(sizes, offy, offx)
    out, _ = _run(img, specs, trace=bool(int(os.environ.get("KERNEL_TRACE", "0"))))
    return out.astype(np.float32)


# revision 17
# speedup vs baseline: 10.6851x; 1.0757x over previous
"""DangoCutouts Trainium2 kernel — two-pass PE-matmul resampler.

Computes reference:
    out[16, 3, 512, 512] =
      [full, gray(full), flip(full), gray(flip(full)), inner_0..11]
    where full = bilinear_resize(img, 4096 -> 512),
          inner_k = bilinear_resize(img[offy_k:+s_k, offx_k:+s_k] -> 512),
          inner_0 additionally grayscaled.

Strategy (8 NeuronCores, data-parallel over output rows):
  Core c computes output rows [64c, 64c+64) of all 16 outputs.
  13 distinct resamples (full + 12 inner). Per resample, per core:
    1. Row gather (dma_gather, SWDGE): T[128, 3, w_al] f32 where
       partition p = y-slot (p<64: y0-row of out row p; p>=64: y1-row),
       free q-slot = channel. One gather per resample (elem = w_al).
    2. Pass 1 (PE, bf16; T cast f32->bf16 split across DVE/ACT): per
       128-wide x-chunk (127-stride so each output column's source
       pair stays within one chunk):
         out1[x, (c,i)] = T[:, c, chunk]^T @ Wy   (Wy = [y-slot, i]
       bilinear row weights, block-diagonal).  This fuses the row
       lerp and transposes x onto partitions in one step.
    3. out1 PSUM -> SBUF bf16 copy (ACT).
    4. Pass 2 (PE, bf16): final[(c,i), j] += out1_chunk^T @ Wx_chunk
       where Wx_chunk[x, j] holds (1-wx_j)/wx_j at the source pair
       rows.  Each j is written by exactly one chunk (no PSUM
       accumulation).  The overview resample additionally runs a
       reversed-Wx pass for the flip.
    5. Final PSUM -> SBUF copies, gray variants (DVE), DMA out.

  All gather/weight tables are host-built from the exact float32
  reference arithmetic; wx weights are bf16 (rel tolerance is 2e-2).
"""
import os
import numpy as np
import ml_dtypes

CUT = 512
H = W = 4096
GRAY_W = (0.2989, 0.587, 0.114)
N_INNER = 12
NSPEC = 13          # full + 12 inner
STRIP = 64          # output rows per core
NCORES = 8
SINGLE_PACKET = True
CHUNK_STRIDE = 127  # lhsT x-chunk stride (128-wide chunks, 1 overlap)

_CACHE = {}


# --------------------------------------------------------------------------
# host-side parameter math (replicates reference._crop_resize in float32)
# --------------------------------------------------------------------------

def _bilinear_params(offy, offx, size):
    s = np.float32(size)
    t = (np.arange(CUT, dtype=np.float32) + np.float32(0.5)) * s / np.float32(CUT) \
        - np.float32(0.5)
    y = np.clip(np.float32(offy) + t, np.float32(offy), np.float32(offy) + s - np.float32(1.0))
    x = np.clip(np.float32(offx) + t, np.float32(offx), np.float32(offx) + s - np.float32(1.0))
    y0 = np.floor(y).astype(np.int32)
    x0 = np.floor(x).astype(np.int32)
    y1 = np.minimum(y0 + 1, np.int32(offy) + np.int32(size) - 1)
    x1 = np.minimum(x0 + 1, np.int32(offx) + np.int32(size) - 1)
    wy = (y - y0.astype(np.float32)).astype(np.float32)
    wx = (x - x0.astype(np.float32)).astype(np.float32)
    # match XLA gather out-of-bounds clamp / negative wrap for degenerate inputs
    for a in (y0, y1):
        np.copyto(a, np.where(a < 0, a % H, np.minimum(a, H - 1)))
    for a in (x0, x1):
        np.copyto(a, np.where(a < 0, a % W, np.minimum(a, W - 1)))
    return y0, y1, wy, x0, x1, wx


def _col_window(x0, x1):
    cx0 = int(x0[0])
    w = int(x1[-1]) - cx0 + 1
    w_al = min(max((w + 63) // 64 * 64, 128), W)
    if cx0 + w_al > W:
        cx0 = W - w_al
    return cx0, w_al


def _wrap16(idx):
    """dma_gather idx-table layout: flat idx list wrapped 16-wide,
    replicated across the 8 Q7 cores."""
    idx = np.asarray(idx, np.int16)
    n = len(idx)
    assert n % 16 == 0
    cols = n // 16
    tile = np.zeros((128, cols), np.int16)
    blk = idx.reshape(cols, 16).T
    for g in range(8):
        tile[16 * g:16 * g + 16, :] = blk
    return tile


def _specs_from_inputs(sizes, offy, offx):
    specs = [(0, 0, min(H, W))]
    for k in range(N_INNER):
        specs.append((int(offy[k]), int(offx[k]), max(int(sizes[k]), 0)))
    return specs


def _params(specs):
    out = []
    for (oy, ox, s) in specs:
        y0, y1, wy, x0, x1, wx = _bilinear_params(oy, ox, max(s, 1) if s <= 0 else s)
        cx0, w_al = _col_window(x0, x1)
        out.append(dict(y0=y0, y1=y1, wy=wy, x0=x0, x1=x1, wx=wx, cx0=cx0, w_al=w_al))
    return out


# --------------------------------------------------------------------------
# column-pass chunk plan + weight tables
# --------------------------------------------------------------------------

def _chunk_plan(x0_rel, x1_rel, wx, w_al):
    """Group output columns j into 128-wide lhsT chunks (stride 127) such
    that both bilinear sources of each j live inside its chunk.  Returns
    (chunks, tables): chunks = [(x_k, j0, bw)], tables = [np[128, bw] f32]."""
    n = len(x0_rel)
    xk_of = np.minimum(CHUNK_STRIDE * (x0_rel // CHUNK_STRIDE), w_al - 128)
    xk_of = np.maximum(xk_of, 0)
    chunks, tables = [], []
    j = 0
    while j < n:
        xk = int(xk_of[j])
        j0 = j
        while j < n and int(xk_of[j]) == xk:
            j += 1
        bw = j - j0
        Wt = np.zeros((128, bw), np.float32)
        for t in range(j0, j):
            x0 = int(x0_rel[t]); x1 = int(x1_rel[t]); w = float(wx[t])
            lx0 = x0 - xk
            assert 0 <= lx0 < 128, (x0, xk)
            Wt[lx0, t - j0] += np.float32(1.0) - np.float32(w)
            if x1 == x0 + 1:
                assert x1 - xk < 128
                Wt[x1 - xk, t - j0] += np.float32(w)
            else:
                assert x1 == x0, (x0, x1)
                Wt[lx0, t - j0] += np.float32(w)
        chunks.append((xk, j0, bw))
        tables.append(Wt)
    return chunks, tables


def _wx_geom(params):
    """Build the (hashable) program geometry + the shared bf16 Wx table."""
    all_tables = []
    geom = []
    coff = 0
    for r, p in enumerate(params):
        w_al = p["w_al"]
        gx0 = (p["x0"] - p["cx0"]).astype(np.int64)
        gx1 = (p["x1"] - p["cx0"]).astype(np.int64)
        chunks, tables = _chunk_plan(gx0, gx1, p["wx"], w_al)
        entries = []
        for (xk, j0, bw), Wt in zip(chunks, tables):
            entries.append([xk, j0, bw, coff])
            all_tables.append(Wt)
            coff += bw
        if r == 0:
            # flip pass: same chunk set, reversed j mapping
            fchunks, ftables = _chunk_plan(gx0[::-1], gx1[::-1],
                                           p["wx"][::-1], w_al)
            fmap = {}
            for (xk, j0, bw), Wt in zip(fchunks, ftables):
                fmap[xk] = (j0, bw, coff)
                all_tables.append(Wt)
                coff += bw
            assert set(fmap) == {e[0] for e in entries}
            entries = [e + list(fmap[e[0]]) for e in entries]
        geom.append((p["cx0"], w_al, tuple(tuple(e) for e in entries)))
    wxt = np.concatenate(all_tables, axis=1).astype(ml_dtypes.bfloat16)
    return tuple(geom), wxt


def _core_tables(params, core):
    """Per-core row-gather idx table [128, 13*24] i16 and Wy [128, 13*64] f32."""
    r0 = core * STRIP
    ridx_cols, wy_cols = [], []
    ar = np.arange(STRIP)
    for p in params:
        y0s = p["y0"][r0:r0 + STRIP].astype(np.int64)
        y1s = p["y1"][r0:r0 + STRIP].astype(np.int64)
        idx = np.zeros(384, np.int64)
        for c in range(3):
            idx[c * 128:c * 128 + 64] = c * H + y0s
            idx[c * 128 + 64:c * 128 + 128] = c * H + y1s
        ridx_cols.append(_wrap16(idx))
        wys = p["wy"][r0:r0 + STRIP].astype(np.float32)
        Wy = np.zeros((128, STRIP), np.float32)
        Wy[ar, ar] = np.float32(1.0) - wys
        Wy[ar + STRIP, ar] = wys
        wy_cols.append(Wy)
    return (np.concatenate(ridx_cols, axis=1),
            np.concatenate(wy_cols, axis=1).astype(np.float32))


def _prepare(img, specs):
    params = _params(specs)
    geom, wxt = _wx_geom(params)
    in_maps = []
    for core in range(NCORES):
        ridx_all, wyt_all = _core_tables(params, core)
        in_maps.append({
            "img": img,
            "ridx": ridx_all,
            "wyt": wyt_all,
            "wxt": wxt,
        })
    return geom, in_maps


# --------------------------------------------------------------------------
# device program
# --------------------------------------------------------------------------

def _build_bass(geom, reps=1, ablate=None, pass1_bf16=True, pipe=4):
    import concourse.bacc as bacc
    import concourse.mybir as mybir
    from concourse.tile import TileContext

    f32 = mybir.dt.float32
    bf16 = mybir.dt.bfloat16
    i16 = mybir.dt.int16
    MUL = mybir.AluOpType.mult
    ADD = mybir.AluOpType.add

    wxcols = max(e[3] + e[2] for (_, _, entries) in geom for e in entries)
    wxcols = max(wxcols,
                 max((e[6] + e[5] for (_, _, es) in geom for e in es
                      if len(e) > 4), default=0))

    nc = bacc.Bacc("TRN2", target_bir_lowering=False)

    img = nc.dram_tensor("img", [3, H, W], f32, kind="ExternalInput")
    img_rows = img.rearrange("c h w -> (c h) w")
    ridx = nc.dram_tensor("ridx", [128, NSPEC * 24], i16, kind="ExternalInput")
    wyt = nc.dram_tensor("wyt", [128, NSPEC * 64], f32, kind="ExternalInput")
    wxt = nc.dram_tensor("wxt", [128, wxcols], bf16, kind="ExternalInput")

    out_d = nc.dram_tensor("out", [16, 3, STRIP, CUT], f32, kind="ExternalOutput")
    out_rows = out_d.rearrange("k c i j -> (k c i) j")

    def out_ap(k, c, nch=1):
        base = (k * 3 + c) * STRIP
        return out_rows[base:base + nch * STRIP, :]

    with TileContext(nc) as tc:
        with (
            tc.tile_pool(name="const", bufs=1) as cpool,
            tc.tile_pool(name="tdata", bufs=2) as tpool,
            tc.tile_pool(name="u", bufs=pipe + 2) as upool,
            tc.tile_pool(name="otiles", bufs=2) as opool,
            tc.tile_pool(name="p1", bufs=max(2, pipe), space="PSUM") as p1pool,
            tc.tile_pool(name="fin", bufs=(1 if pipe > 2 else 2), space="PSUM") as fpool,
            tc.tile_pool(name="flip", bufs=1, space="PSUM") as flpool,
        ):
            ridx_t = cpool.tile([128, NSPEC * 24], i16)
            nc.sync.dma_start(out=ridx_t[:], in_=ridx[:])
            wyt_t = cpool.tile([128, NSPEC * 64], f32)
            nc.sync.dma_start(out=wyt_t[:], in_=wyt[:])
            wxt_t = cpool.tile([128, wxcols], bf16)
            nc.sync.dma_start(out=wxt_t[:], in_=wxt[:])
            if pass1_bf16:
                wyt_b = cpool.tile([128, NSPEC * 64], bf16)
                nc.vector.tensor_copy(wyt_b[:], wyt_t[:])

            def writeout(O01, O2, kout, gray=False, gray_only=False):
                if not gray_only:
                    nc.sync.dma_start(out=out_ap(kout, 0, nch=2), in_=O01[:])
                    nc.sync.dma_start(out=out_ap(kout, 2), in_=O2[:])
                if gray or gray_only:
                    kg = kout + 1 if not gray_only else kout
                    ch1 = opool.tile([64, CUT], f32, tag="ch1")
                    nc.scalar.copy(out=ch1[:], in_=O01[64:128, :])
                    g = opool.tile([64, CUT], f32, tag="gray")
                    nc.scalar.mul(out=g[:], in_=O01[:64, :], mul=float(GRAY_W[0]))
                    nc.vector.scalar_tensor_tensor(out=g[:], in0=ch1[:],
                                                   scalar=float(GRAY_W[1]), in1=g[:],
                                                   op0=MUL, op1=ADD)
                    nc.vector.scalar_tensor_tensor(out=g[:], in0=O2[:],
                                                   scalar=float(GRAY_W[2]), in1=g[:],
                                                   op0=MUL, op1=ADD)
                    for c in range(3):
                        nc.sync.dma_start(out=out_ap(kg, c), in_=g[:])

            PIPE = pipe   # chunks of pass1 issued ahead of each pass2

            for _rep in range(reps):
                for r, (cx0, w_al, entries) in enumerate(geom):
                    T = tpool.tile([128, 3, w_al], f32, tag="T")
                    if ablate == "hwdma":
                        img_hcw = img.rearrange("c h w -> h c w")
                        nc.sync.dma_start(
                            out=T[:], in_=img_hcw[0:128, :, cx0:cx0 + w_al])
                    else:
                        nc.gpsimd.dma_gather(
                            out_ap=T[:],
                            in_ap=img_rows[:, cx0:cx0 + w_al],
                            idxs_ap=ridx_t[:, r * 24:r * 24 + 24],
                            num_idxs=384,
                            num_idxs_reg=384,
                            elem_size=w_al,
                            elem_step=W,
                            single_packet=SINGLE_PACKET,
                        )
                    if ablate == "nocompute" and _rep > 0:
                        continue
                    if pass1_bf16:
                        Tb = tpool.tile([128, 3, w_al], bf16, tag="Tb")
                        # split the f32->bf16 cast across DVE and ACT
                        nc.vector.tensor_copy(Tb[:, 0:2, :], T[:, 0:2, :])
                        nc.scalar.copy(out=Tb[:, 2, :], in_=T[:, 2, :])
                        Tmm = Tb
                    else:
                        Tmm = T
                    finalP = fpool.tile([128, CUT], f32, space="PSUM", tag="fP")
                    finalP2 = fpool.tile([64, CUT], f32, space="PSUM", tag="fP2")
                    is_ovw = len(entries[0]) > 4
                    if is_ovw:
                        flipP = flpool.tile([128, CUT], f32, space="PSUM", tag="xP")
                        flipP2 = flpool.tile([64, CUT], f32, space="PSUM", tag="xP2")
                    wy_sl = (wyt_b if pass1_bf16 else wyt_t)[:, r * STRIP:(r + 1) * STRIP]

                    def pass2(ent, U):
                        xk, j0, bw, coff = ent[:4]
                        nc.tensor.matmul(out=finalP[:, j0:j0 + bw],
                                         lhsT=U[:, 0:128],
                                         rhs=wxt_t[:, coff:coff + bw],
                                         start=True, stop=True)
                        nc.tensor.matmul(out=finalP2[:, j0:j0 + bw],
                                         lhsT=U[:, 128:192],
                                         rhs=wxt_t[:, coff:coff + bw],
                                         start=True, stop=True)
                        if is_ovw:
                            fj0, fbw, fcoff = ent[4:]
                            nc.tensor.matmul(out=flipP[:, fj0:fj0 + fbw],
                                             lhsT=U[:, 0:128],
                                             rhs=wxt_t[:, fcoff:fcoff + fbw],
                                             start=True, stop=True)
                            nc.tensor.matmul(out=flipP2[:, fj0:fj0 + fbw],
                                             lhsT=U[:, 128:192],
                                             rhs=wxt_t[:, fcoff:fcoff + fbw],
                                             start=True, stop=True)

                    staged = []
                    for ci, ent in enumerate(entries):
                        xk = ent[0]
                        P1 = p1pool.tile([128, 192], f32, space="PSUM", tag="P1")
                        for c in range(3):
                            nc.tensor.matmul(out=P1[:, 64 * c:64 * (c + 1)],
                                             lhsT=Tmm[:, c, xk:xk + 128],
                                             rhs=wy_sl, start=True, stop=True)
                        U = upool.tile([128, 192], bf16, tag="U")
                        if ci % 2 == 0:
                            nc.scalar.copy(out=U[:], in_=P1[:])
                        else:
                            nc.vector.tensor_copy(U[:], P1[:])
                        staged.append((ent, U))
                        if len(staged) > PIPE:
                            pass2(*staged.pop(0))
                    for s in staged:
                        pass2(*s)

                    O01 = opool.tile([128, CUT], f32, tag="O01")
                    O2 = opool.tile([64, CUT], f32, tag="O2")
                    nc.scalar.copy(out=O01[:], in_=finalP[:])
                    nc.scalar.copy(out=O2[:], in_=finalP2[:])
                    if r == 0:
                        writeout(O01, O2, 0, gray=True)
                        F01 = opool.tile([128, CUT], f32, tag="F01")
                        F2 = opool.tile([64, CUT], f32, tag="F2")
                        nc.scalar.copy(out=F01[:], in_=flipP[:])
                        nc.scalar.copy(out=F2[:], in_=flipP2[:])
                        writeout(F01, F2, 2, gray=True)
                    else:
                        writeout(O01, O2, 3 + r, gray_only=(r == 1))
    return nc


# --------------------------------------------------------------------------
# entry point
# --------------------------------------------------------------------------

def _run(img, specs, trace=False):
    from concourse.bass_utils import run_bass_kernel_spmd

    geom, in_maps = _prepare(img, specs)

    if geom in _CACHE:
        nc = _CACHE[geom]
    else:
        nc = _build_bass(geom)
        nc.compile()
        _CACHE[geom] = nc

    r = run_bass_kernel_spmd(nc, in_maps, core_ids=list(range(NCORES)),
                             trace=trace)
    strips = [r.results[c]["out"] for c in range(NCORES)]
    out = np.concatenate(strips, axis=2)
    return out, r


def kernel(**inputs):
    img = np.ascontiguousarray(np.asarray(inputs["input"], np.float32)[0])
    sizes = np.asarray(inputs["sizes"])
    offy = np.asarray(inputs["offy"])
    offx = np.asarray(inputs["offx"])
    specs = _specs_from_inputs(sizes, offy, offx)
    out, _ = _run(img, specs, trace=bool(int(os.environ.get("KERNEL_TRACE", "0"))))
    return out.astype(np.float32)
